# revision 1
# baseline (speedup 1.0000x reference)
"""Trainium2 Bass kernel for nn_GRULayer (Keras GRU, reset_after=True, Masking(0)).

Data-parallel over batch: 8 cores x 32 rows, weights replicated, the time scan
local per core. One serial scan chain of T=512 steps per core; the per-step
serial dependency cycle is minimized:

  - "w = 1-z" trick: z-path weights/biases/mask are negated on the host, so a
    sigmoid yields w = 1-z directly and the update is h' = h + w*(c-h).
  - linear recurrence split: U*h(t) = U*h(t-1) + U*(-w@h) + U*(w@c), so the
    only post-tanh work on the critical path is wc = w*c followed by the
    U*wc matmuls; x-projections, biases (tiny one-hot matmuls), the mask
    term, U*h(t-1), and U*wh' are all accumulated into the next step's PSUM
    tile off the critical path.
  - gate math in bf16; state h kept in bf16; recurrent/projection matmuls in
    bf16 with f32 PSUM accumulation.
  - the h-block x-projection is done chunked (16 steps per chunk) and the
    tanh pre-activation add reads it straight from PSUM.
  - off-critical elementwise ops (wc, state updates) run on the GPSIMD/Pool
    engine (SBUF-only operands - a backend requirement).
"""

import os
import sys

sys.path.insert(0, "/opt/trn_rl_repo")

import ml_dtypes
import numpy as np

import concourse.bass as bass
import concourse.mybir as mybir
import concourse.tile as tile
from concourse import bacc
from concourse.bass_utils import run_bass_kernel_spmd
from concourse.masks import make_identity

B, T, D, U = 256, 512, 256, 256
NCORES = 8
BLOC = B // NCORES  # 32
KT = D // 128  # 2
KU = U // 128  # 2
CH = 16  # steps per input chunk
NCH = T // CH
MASK_BIG = 30.0

F32 = mybir.dt.float32
BF16 = mybir.dt.bfloat16
SIG = mybir.ActivationFunctionType.Sigmoid
TANH = mybir.ActivationFunctionType.Tanh
ACOPY = mybir.ActivationFunctionType.Copy

LAST_RESULTS = None


def _build_program():
    nc = bacc.Bacc(
        "TRN2", target_bir_lowering=False, debug=False, num_devices=NCORES
    )

    codesT_d = nc.dram_tensor("codesT", [KT, 128, T * BLOC], BF16, kind="ExternalInput")
    m3_d = nc.dram_tensor("m3", [3, T * 2 * BLOC], BF16, kind="ExternalInput")
    wzr_d = nc.dram_tensor("wzr", [KT, 128, 2 * U], BF16, kind="ExternalInput")
    wh_d = nc.dram_tensor("wh", [KT, 128, U], BF16, kind="ExternalInput")
    uk_d = nc.dram_tensor("uk", [KU, 128, 3 * U], BF16, kind="ExternalInput")
    br2_d = nc.dram_tensor("br2", [2, 128], BF16, kind="ExternalInput")
    bh2_d = nc.dram_tensor("bh2", [2, 128], BF16, kind="ExternalInput")
    bwm3_d = nc.dram_tensor("bwm3", [3, 128], BF16, kind="ExternalInput")
    b0h2_d = nc.dram_tensor("b0h2", [2, 128], BF16, kind="ExternalInput")
    oh2_d = nc.dram_tensor("oh2", [2, 2 * BLOC], BF16, kind="ExternalInput")
    out_d = nc.dram_tensor("out", [KU, BLOC, 128], F32, kind="ExternalOutput")

    with tile.TileContext(nc) as tc:
        with (
            tc.tile_pool(name="const", bufs=1) as const,
            tc.tile_pool(name="cin", bufs=3) as cin,
            tc.tile_pool(name="gate", bufs=2) as gate,
            tc.tile_pool(name="state", bufs=1) as state,
            tc.tile_pool(name="psa", bufs=3, space="PSUM") as psa_pool,
            tc.tile_pool(name="psh", bufs=2, space="PSUM") as psh_pool,
            tc.tile_pool(name="pst", bufs=1, space="PSUM") as pst_pool,
        ):
            # ---- constants ----
            wzr_sb = [const.tile([128, 2 * U], BF16, tag=f"wzr{k}", name=f"wzr{k}") for k in range(KT)]
            wh_sb = [const.tile([128, U], BF16, tag=f"wh{k}", name=f"wh{k}") for k in range(KT)]
            uk_sb = [const.tile([128, 3 * U], BF16, tag=f"uk{k}", name=f"uk{k}") for k in range(KU)]
            for k in range(KT):
                nc.sync.dma_start(out=wzr_sb[k], in_=wzr_d[k])
                nc.sync.dma_start(out=wh_sb[k], in_=wh_d[k])
            for k in range(KU):
                nc.sync.dma_start(out=uk_sb[k], in_=uk_d[k])
            br2_sb = const.tile([2, 128], BF16, tag="br2")
            bh2_sb = const.tile([2, 128], BF16, tag="bh2")
            bwm3_sb = const.tile([3, 128], BF16, tag="bwm3")
            b0h2_sb = [const.tile([1, 128], BF16, tag=f"b0h{m}", name=f"b0h{m}") for m in range(2)]
            oh2_sb = const.tile([2, 2 * BLOC], BF16, tag="oh2")
            nc.sync.dma_start(out=br2_sb, in_=br2_d[:])
            nc.sync.dma_start(out=bh2_sb, in_=bh2_d[:])
            nc.sync.dma_start(out=bwm3_sb, in_=bwm3_d[:])
            for m in range(2):
                nc.sync.dma_start(out=b0h2_sb[m], in_=b0h2_d[m : m + 1, :])
            nc.sync.dma_start(out=oh2_sb, in_=oh2_d[:])
            onesC = const.tile([1, CH * BLOC], BF16, tag="onesC")
            nc.vector.memset(onesC, 1.0)
            ident = const.tile([128, 128], F32, tag="ident")
            make_identity(nc, ident)

            # ---- state ----
            h16 = state.tile([128, KU, BLOC], BF16, tag="h16")
            nc.vector.memset(h16, 0.0)

            chunks = {}
            psa_by_t = {}
            xhs_by_t = {}

            def ensure_chunk(c):
                if c in chunks or c >= NCH:
                    return
                ct = [cin.tile([128, CH * BLOC], BF16, tag=f"ct{k}", name=f"ct{k}") for k in range(KT)]
                for k in range(KT):
                    nc.sync.dma_start(
                        out=ct[k], in_=codesT_d[k, :, c * CH * BLOC : (c + 1) * CH * BLOC]
                    )
                m3c = cin.tile([3, CH * 2 * BLOC], BF16, tag="m3c")
                nc.sync.dma_start(
                    out=m3c, in_=m3_d[:, c * CH * 2 * BLOC : (c + 1) * CH * 2 * BLOC]
                )
                # chunked h-block x-projection (+ b0h)
                psH = psh_pool.tile([128, KU, CH * BLOC], F32, tag="psH")
                for m in range(KU):
                    for k in range(KT):
                        nc.tensor.matmul(
                            psH[:, m],
                            wh_sb[k][:, m * 128 : (m + 1) * 128],
                            ct[k],
                            start=(k == 0),
                            stop=False,
                        )
                    nc.tensor.matmul(
                        psH[:, m], b0h2_sb[m], onesC, start=False, stop=True
                    )
                chunks[c] = (ct, m3c, psH)

            def recmm(psA, rhs, mtiles, stop_last=False):
                # U-matmuls of rhs [128, KU, BLOC] into psA m-tile slices
                for m in mtiles:
                    for k in range(KU):
                        nc.tensor.matmul(
                            psA[:, m],
                            uk_sb[k][:, m * 128 : (m + 1) * 128],
                            rhs[:, k],
                            start=False,
                            stop=(stop_last and m == mtiles[-1] and k == KU - 1),
                            skip_group_check=True,
                        )

            def emit_xproj(t):
                # x-projection + bias/mask for the z/r blocks of step t, plus
                # the U*h(t-2) early recurrent contribution
                c = t // CH
                ensure_chunk(c)
                ct, m3c, _ = chunks[c]
                psA = psa_pool.tile([128, 6, BLOC], F32, tag="psA", name="psA")
                col = (t % CH) * BLOC
                for m in range(4):
                    for k in range(KT):
                        nc.tensor.matmul(
                            psA[:, m],
                            wzr_sb[k][:, m * 128 : (m + 1) * 128],
                            ct[k][:, col : col + BLOC],
                            start=(m == 0 and k == 0),
                            stop=False,
                            skip_group_check=True,
                        )
                mcol = (t % CH) * 2 * BLOC
                # w-tiles: bias + mask (K=3: [-bw0, -bw1 one-hot | -30*(1-m)])
                nc.tensor.matmul(
                    psA[:, 0:2],
                    bwm3_sb,
                    m3c[:, mcol : mcol + 2 * BLOC],
                    start=False,
                    stop=False,
                    skip_group_check=True,
                )
                # r-tiles: bias (K=2 one-hot)
                nc.tensor.matmul(
                    psA[:, 2:4],
                    br2_sb,
                    oh2_sb,
                    start=False,
                    stop=False,
                    skip_group_check=True,
                )
                # hh-tiles: b1h bias (K=2 one-hot)
                nc.tensor.matmul(
                    psA[:, 4:6],
                    bh2_sb,
                    oh2_sb,
                    start=False,
                    stop=False,
                    skip_group_check=True,
                )
                # early recurrent part: U * h(prev state as of emission)
                if t > 0:
                    recmm(psA, h16, (2, 3, 0, 1, 4, 5))
                psa_by_t[t] = psA

            emit_xproj(0)
            xh0 = gate.tile([128, 2, BLOC], BF16, tag="xhN", name="xh0")
            nc.vector.tensor_copy(out=xh0, in_=chunks[0][-1][:, :, 0:BLOC])
            xhs_by_t[0] = xh0
            for t in range(T):
                if t % CH == CH // 2:
                    ensure_chunk(t // CH + 1)
                if t + 1 < T:
                    emit_xproj(t + 1)

                psA = psa_by_t.pop(t)
                psN = psa_by_t.get(t + 1)
                _, _, psH = chunks[t // CH]
                col = (t % CH) * BLOC

                # ---- gates ----
                sig = gate.tile([128, 4, BLOC], BF16, tag="sig")
                nc.scalar.activation(out=sig, in_=psA[:, 0:4], func=SIG)

                t1 = gate.tile([128, 2, BLOC], BF16, tag="t1")
                nc.vector.tensor_mul(out=t1, in0=psA[:, 4:6], in1=sig[:, 2:4])
                q = gate.tile([128, 2, BLOC], BF16, tag="q")
                nc.vector.tensor_add(out=q, in0=t1, in1=xhs_by_t.pop(t))
                # evacuate next step's xh from the chunk PSUM (ready early)
                if t + 1 < T:
                    cN = (t + 1) // CH
                    ensure_chunk(cN)
                    psHN = chunks[cN][-1]
                    ncol = ((t + 1) % CH) * BLOC
                    xhN = gate.tile([128, 2, BLOC], BF16, tag="xhN")
                    with tc.high_priority(offset=-(1 << 20)):
                        nc.vector.tensor_copy(
                            out=xhN, in_=psHN[:, :, ncol : ncol + BLOC]
                        )
                    xhs_by_t[t + 1] = xhN

                # wh' = (-w) * h(t-1): feeds U*wh' into psA(t+1) (Pool)
                wh = gate.tile([128, 2, BLOC], BF16, tag="wh")
                nc.vector.scalar_tensor_tensor(
                    out=wh,
                    in0=sig[:, 0:2],
                    scalar=-1.0,
                    in1=h16,
                    op0=mybir.AluOpType.mult,
                    op1=mybir.AluOpType.mult,
                )
                if psN is not None:
                    recmm(psN, wh, (2, 3, 0, 1, 4, 5))

                cc = gate.tile([128, 2, BLOC], BF16, tag="cc")
                nc.scalar.activation(out=cc, in_=q, func=TANH)

                # wc = w * c: the only post-tanh work on the critical path
                wc = gate.tile([128, 2, BLOC], BF16, tag="wc")
                nc.gpsimd.tensor_mul(out=wc, in0=sig[:, 0:2], in1=cc)
                if psN is not None:
                    recmm(psN, wc, (2, 3, 0, 1, 4, 5), stop_last=True)

                # state update (off critical path)
                nc.gpsimd.tensor_add(out=h16, in0=h16, in1=wh)
                nc.gpsimd.tensor_add(out=h16, in0=h16, in1=wc)

            # ---- epilogue: h16 -> out [KU, BLOC, 128] f32 ----
            h32f = gate.tile([128, KU * BLOC], F32, tag="h32f")
            nc.vector.tensor_copy(out=h32f, in_=h16.rearrange("p k b -> p (k b)"))
            pt = pst_pool.tile([KU * BLOC, 128], F32, tag="pt")
            nc.tensor.transpose(pt, h32f, ident)
            hT = gate.tile([KU * BLOC, 128], F32, tag="hT")
            nc.vector.tensor_copy(out=hT, in_=pt)
            nc.sync.dma_start(out=out_d.rearrange("k b u -> (k b) u"), in_=hT)

    nc.compile()
    return nc


_NC_CACHE = None


def _get_program():
    global _NC_CACHE
    if _NC_CACHE is None:
        _NC_CACHE = _build_program()
    return _NC_CACHE


def _host_prep_shared(W, Uk, b):
    bf = ml_dtypes.bfloat16
    wzr = np.concatenate([-W[:, :U], W[:, U : 2 * U]], axis=1)  # [D, 2U]
    wzr = np.ascontiguousarray(wzr.reshape(KT, 128, 2 * U).astype(bf))
    wh = np.ascontiguousarray(W[:, 2 * U :].reshape(KT, 128, U).astype(bf))
    uk = np.concatenate([-Uk[:, :U], Uk[:, U :]], axis=1)
    uk = np.ascontiguousarray(uk.reshape(KU, 128, 3 * U).astype(bf))
    bzr = b[0] + b[1]
    br2 = np.ascontiguousarray(bzr[U : 2 * U].reshape(2, 128).astype(bf))
    bh2 = np.ascontiguousarray(b[1, 2 * U :].reshape(2, 128).astype(bf))
    bwm3 = np.stack(
        [
            -bzr[0:128],
            -bzr[128:256],
            np.full(128, -MASK_BIG, np.float32),
        ]
    )
    bwm3 = np.ascontiguousarray(bwm3.astype(bf))
    b0h2 = np.ascontiguousarray(b[0, 2 * U :].reshape(2, 128).astype(bf))
    oh2 = np.zeros((2, 2, BLOC), np.float32)
    oh2[0, 0] = 1.0
    oh2[1, 1] = 1.0
    oh2 = np.ascontiguousarray(oh2.reshape(2, 2 * BLOC).astype(bf))
    return dict(wzr=wzr, wh=wh, uk=uk, br2=br2, bh2=bh2, bwm3=bwm3, b0h2=b0h2, oh2=oh2)


def _host_prep_shard(sh):
    """sh: [BLOC, T, D] float32 -> codesT, m3"""
    bf = ml_dtypes.bfloat16
    mask = np.any(sh != 0.0, axis=-1)  # [BLOC, T]
    codesT = np.ascontiguousarray(
        sh.transpose(2, 1, 0).reshape(KT, 128, T * BLOC).astype(bf)
    )
    m3 = np.zeros((3, T, 2, BLOC), np.float32)
    m3[0, :, 0, :] = 1.0
    m3[1, :, 1, :] = 1.0
    inv = 1.0 - mask.T.astype(np.float32)  # [T, BLOC]
    m3[2, :, 0, :] = inv
    m3[2, :, 1, :] = inv
    m3 = np.ascontiguousarray(m3.reshape(3, T * 2 * BLOC).astype(bf))
    return codesT, m3


def kernel(codes: np.ndarray, W: np.ndarray, Uk: np.ndarray, b: np.ndarray):
    codes = np.asarray(codes, dtype=np.float32)
    W = np.asarray(W, dtype=np.float32)
    Uk = np.asarray(Uk, dtype=np.float32)
    b = np.asarray(b, dtype=np.float32)

    shared = _host_prep_shared(W, Uk, b)
    nc = _get_program()
    in_maps = []
    for c in range(NCORES):
        codesT, m3 = _host_prep_shard(codes[c * BLOC : (c + 1) * BLOC])
        in_maps.append({"codesT": codesT, "m3": m3, **shared})

    global LAST_RESULTS
    LAST_RESULTS = run_bass_kernel_spmd(
        nc,
        in_maps,
        list(range(NCORES)),
        trace=False,
    )
    outs = [r["out"].transpose(1, 0, 2).reshape(BLOC, U) for r in LAST_RESULTS.results]
    return np.concatenate(outs, axis=0).astype(np.float32)

